# revision 15
# baseline (speedup 1.0000x reference)
"""Multi-head attention (B=2, S=2048, D=1024, H=16) on 8 Trainium2 cores.

Sharding: data-parallel over batch (2) x tensor-parallel over head groups (4).
Core c handles batch b = c//4 and heads [g*4, g*4+4) where g = c%4.

Per-core dataflow (all matmuls in float32r: fp32 operands truncated to FP22
multiplies, fp32 accumulate - 4x faster than true fp32 on the PE):
  V   = x_v @ Wv_g^T + bv     (s on partitions, dk free), then V1 = [V*m | m]
                              per head, where m is the 0/1 key mask column
  K^T = Wk_g @ x_k^T          (dk on partitions, s free)
  per q-chunk of 512 (projection of Q interleaved with attention so the
  attention pipeline starts as soon as the first Q columns are projected):
    Q^T[:, qc] = Wq_g @ x_q^T[:, qc]
    per head:
      S^T tiles = K^T_h.T-chunks @ Q^T_h     ((k=128) x (q=512) PSUM tiles)
      P^T = exp(S^T / 8)                      (ACT, PSUM->SBUF)
      [ctx^T ; denom] = sum_k V1_h[k].T @ P^T[k]   (65 x 512 PSUM accum;
                              row 64 = sum of unmasked exp = softmax denom)
      ctx_n^T = ctx^T * bcast(1/denom)        (matmul broadcast + DVE mult)
    out_partial[qc] = ctx_n^T.T @ Wo_g^T      ((q=128) x 1024 chunks -> DRAM)

Host: shards + pre-transposes inputs, sums the 4 head-group partials per batch,
adds bo.  Masked-out keys are excluded exactly (mask column zeros them), so
results match reference softmax(-1e9-masked) semantics.
"""

import numpy as np

import concourse.bass as bass
import concourse.tile as tile
from concourse import bacc, mybir
from concourse.bass_utils import run_bass_kernel_spmd

F32R = mybir.dt.float32r
F32 = mybir.dt.float32
EXP = mybir.ActivationFunctionType.Exp

B, S, D = 2, 2048, 1024
HEADS, DK = 16, 64
G = 4                 # head-groups (tensor parallel factor)
HPG = HEADS // G      # 4 heads per group
DH = HPG * DK         # 256 head-dims per group
NCORES = 8
NT = D // 128         # 8 contraction tiles over d_model
NU = S // 128         # 16 s-chunks of 128 (k-position tiles)
NQC = S // 512        # 4 q-chunks of 512

_cached = {}


def _emit(nc, tc, pools, dram, rep):
    (singles, xpool, xqpool, ppool, opool, rpool, big_ps, ctxp, auxp) = pools
    (xkT, xqT, xvT, wqT, wkT, wvT, woT, bq2, bk2, bvr, m01, onec, out) = dram

    def resident(name, shape, dt=F32R):
        return singles.tile(shape, dt, tag=name, name=f"{name}_r{rep}")

    # ---- resident tensors; wv loads first (first consumer) ----
    wv_sb = [resident(f"wv{t}", [128, DH]) for t in range(NT)]
    for t in range(NT):
        nc.scalar.dma_start(out=wv_sb[t][:], in_=wvT[t * 128:(t + 1) * 128, :])
    bq_sb = resident("bq_sb", [128, 2], F32)
    bk_sb = resident("bk_sb", [128, 2], F32)
    bvr_sb = resident("bvr_sb", [1, DH])
    m01_sb = resident("m01_sb", [128, NU], F32)
    ones1 = resident("ones1", [1, 128])
    ones64 = ones1[0:1, 0:64]
    kT_sb = [resident(f"kT{m}", [128, S]) for m in range(2)]
    qT_sb = [resident(f"qT{m}", [128, S]) for m in range(2)]
    ctxT_sb = [resident(f"ctxT{m}", [128, S]) for m in range(2)]
    v_all = resident("v_all", [128, NU, HPG * 65])
    v4 = v_all.rearrange("p u (h e) -> p u h e", e=65)

    # ---- V projection: V = x @ W^T + bv, then V1 = [V*m | m] ----
    # 16 u-chunks of (128,256) packed into 2 big (6 u) + 2 small (2 u)
    # psum tiles; one matmul accumulation group per PSUM bank (2 u's),
    # started by the first u of the bank, stopped by the bias matmul of
    # the second.
    vb = [big_ps.tile([128, 1536], F32, tag="big", name=f"vb{i}_r{rep}")
          for i in range(2)]
    vs = [ctxp.tile([128, 512], F32, tag="ctx", name=f"vs0_r{rep}"),
          auxp.tile([128, 512], F32, tag="aux", name=f"vs1_r{rep}")]

    def v_slice(u):
        if u < 12:
            return vb[u // 6][:, (u % 6) * 256:(u % 6 + 1) * 256]
        return vs[(u - 12) // 2][:, ((u - 12) % 2) * 256:((u - 12) % 2 + 1) * 256]

    for t in range(NT):
        xt = xpool.tile([128, S], F32R, tag="x", name=f"xv{t}_r{rep}")
        nc.sync.dma_start(out=xt[:], in_=xvT[t * 128:(t + 1) * 128, :])
        for u in range(NU):
            nc.tensor.matmul(
                v_slice(u), xt[:, u * 128:(u + 1) * 128], wv_sb[t][:],
                start=(t == 0 and u % 2 == 0), stop=False,
                skip_group_check=True)
    # small constants land while the xv stream drains
    nc.scalar.dma_start(out=bq_sb[:], in_=bq2)
    nc.scalar.dma_start(out=bk_sb[:], in_=bk2)
    nc.scalar.dma_start(out=bvr_sb[:], in_=bvr)
    nc.scalar.dma_start(out=m01_sb[:], in_=m01)
    nc.scalar.dma_start(out=ones1[:], in_=onec)
    # mask columns of V1 (the "ones column" that builds softmax denoms)
    for h in range(HPG):
        nc.vector.tensor_copy(
            out=v4[:, :, h, 64:65],
            in_=m01_sb[:].rearrange("p (u o) -> p u o", o=1),
        )
    for u in range(NU):
        nc.tensor.matmul(
            v_slice(u), ones1[:], bvr_sb[:],
            start=False, stop=(u % 2 == 1), skip_group_check=True)
    for u in range(NU):
        nc.vector.tensor_scalar_mul(
            out=v4[:, u, :, 0:64],
            in0=v_slice(u).rearrange("p (h e) -> p h e", e=64),
            scalar1=m01_sb[:, u:u + 1])

    # ---- Q weights + first Q-chunk projection go ahead of K so the xq(qc0)
    # DMA lands between the xv and xk streams and the aux PSUM slot cycles
    # vs1 -> qp0x -> ksml1 without blocking ----
    wq_sb = [resident(f"wq{t}", [128, DH]) for t in range(NT)]
    for t in range(NT):
        nc.scalar.dma_start(out=wq_sb[t][:], in_=wqT[t * 128:(t + 1) * 128, :])

    def qproj(qc):
        # Q-projection for one q-chunk: m-tiles sequentially through the one
        # aux PSUM slot; the 8 xq tiles of this chunk stay resident across
        # both m passes.
        qsl = slice(qc * 512, (qc + 1) * 512)
        xts = []
        for t in range(NT):
            xt = xqpool.tile([128, 512], F32R, tag="xq", name=f"xq{qc}_{t}_r{rep}")
            nc.sync.dma_start(out=xt[:], in_=xqT[t * 128:(t + 1) * 128, qsl])
            xts.append(xt)
        for m in range(2):
            qp = auxp.tile([128, 512], F32, tag="aux", name=f"qp{qc}_{m}_r{rep}")
            for t in range(NT):
                nc.tensor.matmul(
                    qp[:], wq_sb[t][:, m * 128:(m + 1) * 128], xts[t][:],
                    start=(t == 0), stop=(t == NT - 1))
            nc.vector.tensor_scalar_add(
                out=qT_sb[m][:, qsl], in0=qp[:], scalar1=bq_sb[:, m:m + 1])

    qproj(0)

    # ---- K^T projection: K^T = Wk @ x_k^T ----
    wk_sb = [resident(f"wk{t}", [128, DH]) for t in range(NT)]
    for t in range(NT):
        nc.scalar.dma_start(out=wk_sb[t][:], in_=wkT[t * 128:(t + 1) * 128, :])
    kbig = [big_ps.tile([128, 1536], F32, tag="big", name=f"kb{m}_r{rep}")
            for m in range(2)]
    ksml = [ctxp.tile([128, 512], F32, tag="ctx", name=f"ks0_r{rep}"),
            auxp.tile([128, 512], F32, tag="aux", name=f"ks1_r{rep}")]
    for t in range(NT):
        xt = xpool.tile([128, S], F32R, tag="x", name=f"xk{t}_r{rep}")
        nc.sync.dma_start(out=xt[:], in_=xkT[t * 128:(t + 1) * 128, :])
        for m in range(2):
            lhsT = wk_sb[t][:, m * 128:(m + 1) * 128]
            for i in range(3):
                nc.tensor.matmul(
                    kbig[m][:, i * 512:(i + 1) * 512], lhsT,
                    xt[:, i * 512:(i + 1) * 512],
                    start=(t == 0), stop=(t == NT - 1))
            nc.tensor.matmul(
                ksml[m][:], lhsT, xt[:, 1536:2048],
                start=(t == 0), stop=(t == NT - 1))
    for m in range(2):
        nc.vector.tensor_scalar_add(
            out=kT_sb[m][:, 0:1536], in0=kbig[m][:],
            scalar1=bk_sb[:, m:m + 1])
        nc.vector.tensor_scalar_add(
            out=kT_sb[m][:, 1536:2048], in0=ksml[m][:],
            scalar1=bk_sb[:, m:m + 1])

    # ---- O-proj weights + second primed Q chunk ----
    wo_sb = [resident(f"wo{m}", [128, D]) for m in range(2)]
    for m in range(2):
        nc.scalar.dma_start(out=wo_sb[m][:], in_=woT[m * 128:(m + 1) * 128, :])
    qproj(1)

    # ---- attention: software-pipelined over (qc, h) ----
    # Producer step P(qc,h): 6 S^T matmul batches + exps filling pt tiles.
    # Consumer step C(qc,h): 16 ctx matmuls + normalize, lagging one head so
    # its PE work interleaves with the NEXT head's S^T batches (the PE FIFO
    # never parks on a not-yet-satisfied wait while ready work sits behind).
    state = {}

    def attn_produce(qc, h):
        m, roff = h // 2, (h % 2) * 64
        qsl = slice(qc * 512, (qc + 1) * 512)
        kT_h = kT_sb[m][roff:roff + 64, :]
        qT_h = qT_sb[m][roff:roff + 64, qsl]
        pt = [ppool.tile([128, 4096], F32R, tag="pt",
                         name=f"pt{qc}_{h}_{half}_r{rep}") for half in range(2)]
        batches = []
        for half in range(2):
            for (b0, bsz) in ((0, 3), (3, 3), (6, 2)):
                batches.append((half, b0, bsz))
        state[(qc, h)] = pt

        def emit_batch(i):
            half, b0, bsz = batches[i]
            st = big_ps.tile([128, bsz * 512], F32, tag="big",
                             name=f"st{qc}_{h}_{half}_{b0}_r{rep}")
            for j in range(bsz):
                k = half * 8 + b0 + j
                nc.tensor.matmul(
                    st[:, j * 512:(j + 1) * 512],
                    kT_h[:, k * 128:(k + 1) * 128], qT_h,
                    start=True, stop=True)
            nc.scalar.activation(
                out=pt[half][:, b0 * 512:(b0 + bsz) * 512],
                in_=st[:, 0:bsz * 512], func=EXP, scale=0.125)
        return emit_batch

    def attn_consume_mms(qc, h):
        # generator of ctx matmul emitters, 16 k-chunks
        m, roff = h // 2, (h % 2) * 64
        ctx_ps = ctxp.tile([65, 512], F32, tag="ctx", name=f"ctx{qc}_{h}_r{rep}")
        pt = state[(qc, h)]

        def emit_k(k):
            nc.tensor.matmul(
                ctx_ps[:], v_all[:, k, h * 65:(h + 1) * 65],
                pt[k // 8][:, (k % 8) * 512:(k % 8 + 1) * 512],
                start=(k == 0), stop=(k == NU - 1))
        return ctx_ps, emit_k

    def attn_norm(qc, h, ctx_ps):
        m, roff = h // 2, (h % 2) * 64
        qsl = slice(qc * 512, (qc + 1) * 512)
        cx = rpool.tile([65, 512], F32, tag="cx", name=f"cx{qc}_{h}_r{rep}")
        nc.vector.tensor_copy(out=cx[:], in_=ctx_ps[:])
        rec = rpool.tile([1, 512], F32R, tag="rec", name=f"rc{qc}_{h}_r{rep}")
        nc.vector.reciprocal(out=rec[:], in_=cx[64:65, :])
        bc = auxp.tile([64, 512], F32, tag="aux", name=f"bc{qc}_{h}_r{rep}")
        nc.tensor.matmul(bc[:], ones64[:], rec[:], start=True, stop=True)
        nc.vector.tensor_mul(
            out=ctxT_sb[m][roff:roff + 64, qsl],
            in0=bc[:], in1=cx[0:64, :])

    def oproj(qc):
        # generator of o-proj emitters: 4 sub-chunks x 2 nj halves
        def emit(sc, nj, o_sb):
            qi = qc * 4 + sc
            ops = auxp.tile([128, 512], F32, tag="aux",
                            name=f"op{qi}_{nj}_r{rep}")
            for m_ in range(2):
                nc.tensor.matmul(
                    ops[:], ctxT_sb[m_][:, qi * 128:(qi + 1) * 128],
                    wo_sb[m_][:, nj * 512:(nj + 1) * 512],
                    start=(m_ == 0), stop=(m_ == 1))
            nc.vector.tensor_copy(
                out=o_sb[:, nj * 512:(nj + 1) * 512], in_=ops[:])
            if nj == 1:
                nc.sync.dma_start(out=out[qi * 128:(qi + 1) * 128, :],
                                  in_=o_sb[:])
        return emit

    units = [(qc, h) for qc in range(NQC) for h in range(HPG)]
    prev = None            # (qc, h, ctx_ps, emit_k) being consumed
    odue = []              # pending o-proj emits: (emit, sc, nj, o_sb)
    for idx, (qc, h) in enumerate(units):
        emit_batch = attn_produce(qc, h)
        for i in range(6):
            emit_batch(i)
            if prev is not None:
                pqc, ph, pctx, pemit = prev
                k0 = i * 3
                for k in range(k0, min(k0 + 3, NU)):
                    pemit(k)
                if i == 5:
                    attn_norm(pqc, ph, pctx)
            # drain up to 2 pending o-proj units per batch slot
            for _ in range(2):
                if odue:
                    odue.pop(0)()
        if prev is not None and prev[1] == HPG - 1:
            # previous head closed a q-chunk: queue its o-proj + prime Q
            pqc = prev[0]
            oemit = oproj(pqc)
            for sc in range(4):
                o_sb = opool.tile([128, D], F32, tag="out",
                                  name=f"o{pqc}_{sc}_r{rep}")
                for nj in range(2):
                    odue.append(lambda oe=oemit, s=sc, n=nj, ob=o_sb: oe(s, n, ob))
            if pqc + 2 < NQC:
                qproj(pqc + 2)
        ctx_ps, emit_k = attn_consume_mms(qc, h)
        prev = (qc, h, ctx_ps, emit_k)
    # drain the final head, then fan the last q-chunk's o-proj across the
    # (now idle) big PSUM slots instead of serializing through the aux slot
    pqc, ph, pctx, pemit = prev
    for k in range(NU):
        pemit(k)
        if odue and k % 2 == 1:
            odue.pop(0)()
    attn_norm(pqc, ph, pctx)
    for fn in odue:
        fn()
    for sc in range(4):
        qi = (NQC - 1) * 4 + sc
        o_sb = opool.tile([128, D], F32, tag="out", name=f"o3_{sc}_r{rep}")
        for nj in range(2):
            pool = big_ps if (sc * 2 + nj) % 3 else auxp
            tag = "big" if pool is big_ps else "aux"
            ops = pool.tile([128, 512], F32, tag=tag, name=f"opf{qi}_{nj}_r{rep}")
            for m_ in range(2):
                nc.tensor.matmul(
                    ops[:], ctxT_sb[m_][:, qi * 128:(qi + 1) * 128],
                    wo_sb[m_][:, nj * 512:(nj + 1) * 512],
                    start=(m_ == 0), stop=(m_ == 1))
            nc.vector.tensor_copy(
                out=o_sb[:, nj * 512:(nj + 1) * 512], in_=ops[:])
        nc.sync.dma_start(out=out[qi * 128:(qi + 1) * 128, :], in_=o_sb[:])


def _build_program(reps=1):
    nc = bacc.Bacc("TRN2", target_bir_lowering=False, debug=False,
                   num_devices=NCORES)

    # ---- DRAM I/O (float32r is bit-identical to float32 host-side) ----
    xkT = nc.dram_tensor("xkT", [D, S], F32R, kind="ExternalInput").ap()
    xqT = nc.dram_tensor("xqT", [D, S], F32R, kind="ExternalInput").ap()
    xvT = nc.dram_tensor("xvT", [D, S], F32R, kind="ExternalInput").ap()
    wqT = nc.dram_tensor("wqT", [D, DH], F32R, kind="ExternalInput").ap()
    wkT = nc.dram_tensor("wkT", [D, DH], F32R, kind="ExternalInput").ap()
    wvT = nc.dram_tensor("wvT", [D, DH], F32R, kind="ExternalInput").ap()
    woT = nc.dram_tensor("woT", [DH, D], F32R, kind="ExternalInput").ap()
    bq2 = nc.dram_tensor("bq2", [128, 2], F32, kind="ExternalInput").ap()
    bk2 = nc.dram_tensor("bk2", [128, 2], F32, kind="ExternalInput").ap()
    bvr = nc.dram_tensor("bvr", [1, DH], F32R, kind="ExternalInput").ap()
    m01 = nc.dram_tensor("m01", [128, NU], F32, kind="ExternalInput").ap()
    onec = nc.dram_tensor("onec", [1, 128], F32R, kind="ExternalInput").ap()
    out = nc.dram_tensor("out", [S, D], F32, kind="ExternalOutput").ap()
    dram = (xkT, xqT, xvT, wqT, wkT, wvT, woT, bq2, bk2, bvr, m01, onec, out)

    with tile.TileContext(nc) as tc:
        with (
            nc.allow_low_precision(
                reason="float32r SBUF tiles are bit-identical to fp32; the PE "
                       "truncates to fp22 at multiply regardless"),
            tc.tile_pool(name="singles", bufs=1) as singles,
            tc.tile_pool(name="xpool", bufs=3) as xpool,
            tc.tile_pool(name="xqpool", bufs=10) as xqpool,
            tc.tile_pool(name="ppool", bufs=3) as ppool,
            tc.tile_pool(name="opool", bufs=2) as opool,
            tc.tile_pool(name="rpool", bufs=2) as rpool,
            tc.tile_pool(name="big_ps", bufs=2, space="PSUM") as big_ps,
            tc.tile_pool(name="ctx_ps", bufs=1, space="PSUM") as ctxp,
            tc.tile_pool(name="aux_ps", bufs=1, space="PSUM") as auxp,
        ):
            pools = (singles, xpool, xqpool, ppool, opool, rpool, big_ps,
                     ctxp, auxp)
            for rep in range(reps):
                _emit(nc, tc, pools, dram, rep)

    nc.compile()
    return nc


def _get_program():
    if "nc" not in _cached:
        _cached["nc"] = _build_program()
    return _cached["nc"]


def kernel(query, key, value, mask, Wq, bq, Wk, bk, Wv, bv, Wo, bo):
    query = np.asarray(query, dtype=np.float32)
    key = np.asarray(key, dtype=np.float32)
    value = np.asarray(value, dtype=np.float32)
    mask = np.asarray(mask)
    Wq, bq = np.asarray(Wq, dtype=np.float32), np.asarray(bq, dtype=np.float32)
    Wk, bk = np.asarray(Wk, dtype=np.float32), np.asarray(bk, dtype=np.float32)
    Wv, bv = np.asarray(Wv, dtype=np.float32), np.asarray(bv, dtype=np.float32)
    Wo, bo = np.asarray(Wo, dtype=np.float32), np.asarray(bo, dtype=np.float32)

    nc = _get_program()

    c = np.ascontiguousarray
    in_maps = []
    for core in range(NCORES):
        b, g = core // G, core % G
        sl = slice(g * DH, (g + 1) * DH)
        mk = (mask[b, 0, 0, :] != 0).astype(np.float32)
        in_maps.append({
            "xqT": c(query[b].T), "xkT": c(key[b].T), "xvT": c(value[b].T),
            "wqT": c(Wq[sl, :].T), "wkT": c(Wk[sl, :].T), "wvT": c(Wv[sl, :].T),
            "woT": c(Wo[:, sl].T),
            "bq2": c(bq[sl].reshape(2, 128).T), "bk2": c(bk[sl].reshape(2, 128).T),
            "bvr": c(bv[sl].reshape(1, DH)),
            "m01": c(mk.reshape(NU, 128).T),
            "onec": np.ones((1, 128), dtype=np.float32),
        })

    res = run_bass_kernel_spmd(nc, in_maps, core_ids=list(range(NCORES)))
    _cached["last_results"] = res

    result = np.empty((B, S, D), dtype=np.float32)
    for b in range(B):
        acc = res.results[b * G + 0]["out"].copy()
        for g in range(1, G):
            acc += res.results[b * G + g]["out"]
        result[b] = acc + bo
    return result
